# revision 21
# baseline (speedup 1.0000x reference)
"""Fused attention kernel for TRN2, 8 NeuronCores, data-parallel over batch.

Problem: q = target @ Wq.T + bq ; k = non_target @ Wk.T + bk ;
         v = non_target @ Wv.T + bv ; out = softmax(q k^T) v
Shapes: target/non_target [8, 2048, 1024], W* [1024, 1024], b* [1024].

Math reformulation (per batch, all on one core):
  softmax(q k^T) is row-shift invariant, so terms constant over kv drop:
    S' = T M N^T + 1 w^T,  M = Wq^T Wk,  w = N (Wk^T bq)   (bk drops out)
  Work in the transposed orientation S'^T = N G', with
    G'[d',q] = sum_d M[d,d'] T^T[d,q] + y[d'],  y = Wk^T bq
  P^T = exp(S'^T) unnormalized (scores are small enough that exp cannot
  overflow fp32), r[q] = sum_kv P^T[kv,q], and the V-projection folds into
  the output matmul:
    O[q,e] = ( sum_d Z^T[d,q] Wv^T[d,e] + r[q] bv[e] ) / r[q]
    Z^T[d,q] = sum_kv N[kv,d] P^T[kv,q]
  This uses non_target in both natural (kv-part) and transposed (d-part)
  layouts; N^T is kept resident in SBUF and N natural is re-streamed from
  HBM per q-chunk. Only T, N and Wv ever need transposing (done on the PE
  with identity matmuls, which are exact).
"""

import os
import numpy as np

import concourse.bass as bass
import concourse.mybir as mybir
import concourse.tile as tile
from concourse import bacc
from concourse.bass_utils import run_bass_kernel_spmd
from concourse.masks import make_identity

F32 = mybir.dt.float32
F32R = mybir.dt.float32r

B, SQ, SKV, D = 8, 2048, 2048, 1024
P = 128
QC = 256                 # q-chunk size
NCHUNK = SQ // QC        # 8
DB = D // P              # 8 d-blocks
KVB = SKV // P           # 16 kv-blocks
NCORES = 8

_CACHE = {}


def _build():
    nc = bacc.Bacc()
    tgt = nc.declare_dram_parameter("target", [SQ, D], F32R, isOutput=False)
    ntg = nc.declare_dram_parameter("non_target", [SKV, D], F32R, isOutput=False)
    wqp = nc.declare_dram_parameter("Wq", [D, D], F32R, isOutput=False)
    wkp = nc.declare_dram_parameter("Wk", [D, D], F32R, isOutput=False)
    wvp = nc.declare_dram_parameter("Wv", [D, D], F32R, isOutput=False)
    bqp = nc.declare_dram_parameter("bq", [D], F32, isOutput=False)
    bvp = nc.declare_dram_parameter("bv", [D], F32, isOutput=False)
    outp = nc.declare_dram_parameter("out", [SQ, D], F32, isOutput=True)
    with tile.TileContext(nc) as tc:
        _emit(nc, tc, tgt, ntg, wqp, wkp, wvp, bqp, bvp, outp)
    nc.compile()
    return nc


def _emit(nc, tc, tgt, ntg, wqp, wkp, wvp, bqp, bvp, outp):
    import contextlib
    ctx = contextlib.ExitStack()
    with ctx:
        # ---- pools: PSUM ----
        mp = ctx.enter_context(tc.tile_pool(name="mp", bufs=2, space="PSUM"))

        # ---- small residents ----
        R = ctx.enter_context(tc.tile_pool(name="resident", bufs=1))
        identF = R.tile([P, P], F32)
        make_identity(nc, identF)
        identR = R.tile([P, P], F32R)
        nc.vector.tensor_copy(identR, identF)
        Mt = R.tile([P, DB, D], F32R)       # M: [d in block, d-block, d']
        y_col = R.tile([P, DB], F32)        # y: [d' in block, d'-block]
        bq_col = R.tile([P, DB], F32)       # bq: [e in block, e-block]
        bv_bc = R.tile([P, D], F32)         # bv broadcast to 128 partitions
        nc.sync.dma_start(bq_col, bqp[:].rearrange("(b p) -> p b", p=P))
        bv_bcast_ap = bass.AP(
            tensor=bvp[:].tensor, offset=0,
            ap=[[0, P], [1, D]],
        )
        nc.gpsimd.dma_start(out=bv_bc, in_=bv_bcast_ap)

        # ---- N^T resident + build (overlaps weight DMAs) ----
        Rnt = ctx.enter_context(tc.tile_pool(name="rnt", bufs=1))
        NT = Rnt.tile([P, DB, SKV], F32R)   # N^T: [d' in block, d'-block, kv]

        with tc.tile_pool(name="wtmp", bufs=1) as W, \
             tc.tile_pool(name="nbuild", bufs=8) as NB, \
             tc.tile_pool(name="pp", bufs=4, space="PSUM") as pp:
            Wqt = W.tile([P, DB, D], F32R)
            Wkt = W.tile([P, DB, D], F32R)
            nlds = []
            with tc.high_priority():
                for j in range(KVB):
                    nld = NB.tile([P, D], F32R, tag="nld")
                    eng = nc.sync if j % 2 == 0 else nc.scalar
                    eng.dma_start(nld, ntg[j * P:(j + 1) * P, :])
                    nlds.append(nld)
            nc.sync.dma_start(Wqt, wqp[:].rearrange("(b p) d -> p b d", p=P))
            nc.scalar.dma_start(Wkt, wkp[:].rearrange("(b p) d -> p b d", p=P))
            for j in range(KVB):
                nld = nlds[j]
                for g in range(2):
                    tp = pp.tile([P, 4, P], F32R, tag="pp")
                    for k in range(4):
                        db = 4 * g + k
                        nc.tensor.transpose(tp[:, k, :], nld[:, db * P:(db + 1) * P], identR)
                    dst = NT[:, 4 * g:4 * g + 4, j * P:(j + 1) * P]
                    if g == 0:
                        nc.vector.tensor_copy(dst, tp)
                    else:
                        nc.scalar.activation(dst, tp, mybir.ActivationFunctionType.Copy)
            for db in range(DB):
                for ch in range(2):
                    ps = mp.tile([P, 512], F32, tag="mp")
                    for eb in range(DB):
                        nc.tensor.matmul(
                            ps,
                            Wqt[:, eb, db * P:(db + 1) * P],
                            Wkt[:, eb, ch * 512:(ch + 1) * 512],
                            start=(eb == 0), stop=(eb == DB - 1),
                        )
                    nc.vector.tensor_copy(Mt[:, db, ch * 512:(ch + 1) * 512], ps)
            for ob in range(DB):
                yp = mp.tile([P, 512], F32, tag="mp")
                for eb in range(DB):
                    nc.tensor.matmul(
                        yp[:, 0:1],
                        Wkt[:, eb, ob * P:(ob + 1) * P].bitcast(F32),
                        bq_col[:, eb:eb + 1],
                        start=(eb == 0), stop=(eb == DB - 1),
                    )
                nc.vector.tensor_copy(y_col[:, ob:ob + 1], yp[:, 0:1])

        # ---- Wv^T ----
        Rwv = ctx.enter_context(tc.tile_pool(name="rwv", bufs=1))
        WvT = Rwv.tile([P, DB, D], F32R)    # Wv^T: [d in block, d-block, e]
        with tc.tile_pool(name="wtmp2", bufs=1) as W2, \
             tc.tile_pool(name="pp2", bufs=4, space="PSUM") as pp2:
            Wvt = W2.tile([P, DB, D], F32R)
            nc.sync.dma_start(Wvt, wvp[:].rearrange("(b p) d -> p b d", p=P))
            for eb in range(DB):
                for g in range(2):
                    tp = pp2.tile([P, 4, P], F32R, tag="pp")
                    for k in range(4):
                        db = 4 * g + k
                        nc.tensor.transpose(tp[:, k, :], Wvt[:, eb, db * P:(db + 1) * P], identR)
                    dst = WvT[:, 4 * g:4 * g + 4, eb * P:(eb + 1) * P]
                    if g == 0:
                        nc.vector.tensor_copy(dst, tp)
                    else:
                        nc.scalar.activation(dst, tp, mybir.ActivationFunctionType.Copy)

        # ---- main-loop pools ----
        stream = ctx.enter_context(tc.tile_pool(name="stream", bufs=2))
        sp = ctx.enter_context(tc.tile_pool(name="sp", bufs=2, space="PSUM"))
        zpp = ctx.enter_context(tc.tile_pool(name="zp", bufs=1, space="PSUM"))
        tld = ctx.enter_context(tc.tile_pool(name="tld", bufs=2))
        chk = ctx.enter_context(tc.tile_pool(name="chk", bufs=1))
        ptp = ctx.enter_context(tc.tile_pool(name="ptp", bufs=3))
        osb = ctx.enter_context(tc.tile_pool(name="osb", bufs=2))
        smal = ctx.enter_context(tc.tile_pool(name="smal", bufs=4))

        def emit_tt(c):
            """T^T for chunk c."""
            q0 = c * QC
            TT = chk.tile([P, DB, QC], F32R, tag="tt")
            for qb in range(QC // P):
                tl = tld.tile([P, D], F32R, tag="tload")
                nc.scalar.dma_start(tl, tgt[q0 + qb * P:q0 + (qb + 1) * P, :])
                for g in range(2):
                    tp = mp.tile([P, 512], F32R, tag="mp")
                    tpv = tp.rearrange("p (k c) -> p k c", k=4)
                    for k in range(4):
                        db = 4 * g + k
                        nc.tensor.transpose(tpv[:, k, :], tl[:, db * P:(db + 1) * P], identR)
                    dst = TT[:, 4 * g:4 * g + 4, qb * P:(qb + 1) * P]
                    if g == 0:
                        nc.vector.tensor_copy(dst, tpv)
                    else:
                        nc.scalar.activation(dst, tpv, mybir.ActivationFunctionType.Copy)
            return TT

        def emit_gp(TT):
            Gp = chk.tile([P, DB, QC], F32R, tag="gp")
            for ob in range(DB):
                gp_ps = mp.tile([P, 512], F32, tag="mp")
                for db in range(DB):
                    nc.tensor.matmul(
                        gp_ps[:, 0:QC],
                        Mt[:, db, ob * P:(ob + 1) * P],
                        TT[:, db, :],
                        start=(db == 0), stop=(db == DB - 1),
                    )
                nc.vector.tensor_scalar_add(Gp[:, ob, :], gp_ps[:, 0:QC],
                                            y_col[:, ob:ob + 1])
            return Gp

        TT = emit_tt(0)
        Gp = emit_gp(TT)
        for c in range(NCHUNK):
            q0 = c * QC
            # ---- kv loop: S' -> exp -> racc ; Z pipelined one pair behind ----
            zp = zpp.tile([P, DB, QC], F32)
            nc.vector.memset(zp, 0.0)
            racc2 = chk.tile([P, 2, QC], F32, tag="racc2")
            racc = chk.tile([P, QC], F32, tag="racc")
            NPAIR = KVB // 2
            pts = {}
            nsts = {}
            for jj in range(NPAIR + 1):
                if jj < NPAIR:
                    spt = sp.tile([P, 2, QC], F32)
                    for h in range(2):
                        j = 2 * jj + h
                        for ob in range(DB):
                            nc.tensor.matmul(
                                spt[:, h, :],
                                NT[:, ob, j * P:(j + 1) * P],
                                Gp[:, ob, :],
                                start=(ob == 0), stop=(ob == DB - 1),
                            )
                        nst = stream.tile([P, D], F32R, tag="nstr")
                        nc.sync.dma_start(nst, ntg[j * P:(j + 1) * P, :])
                        nsts[j] = nst
                    pt = ptp.tile([P, 2, QC], F32R, tag="pt")
                    nc.scalar.activation(pt, spt, mybir.ActivationFunctionType.Exp)
                    pts[jj] = pt
                    if jj == 0:
                        nc.vector.tensor_copy(racc2, pt)
                    else:
                        nc.vector.tensor_add(racc2, racc2, pt)
                if jj > 0:
                    zjj = jj - 1
                    pt = pts.pop(zjj)
                    for h in range(2):
                        j = 2 * zjj + h
                        nst = nsts.pop(j)
                        for db in range(DB):
                            nc.tensor.matmul(
                                zp[:, db, :],
                                nst[:, db * P:(db + 1) * P],
                                pt[:, h, :],
                                start=False, stop=(j == KVB - 1),
                                skip_group_check=True,
                            )

            # ---- Z psum -> sbuf (DVE starts as soon as zp completes) ----
            Zs = chk.tile([P, DB, QC], F32R, tag="zs")
            nc.vector.tensor_copy(Zs, zp)
            nc.vector.tensor_add(racc, racc2[:, 0, :], racc2[:, 1, :])

            # ---- next chunk's TT transposes keep PE busy while copies drain ----
            TT_next = Gp_next = None
            if c + 1 < NCHUNK:
                TT_next = emit_tt(c + 1)

            # ---- r chain: (PE transpose, DVE reduce, recip) ----
            rr_cols = []
            for qb in range(QC // P):
                tp = mp.tile([P, 512], F32, tag="mp")
                nc.tensor.transpose(tp[:, 0:P], racc[:, qb * P:(qb + 1) * P], identF)
                rcol = smal.tile([P, 1], F32, tag="rcol")
                nc.vector.reduce_sum(out=rcol, in_=tp[:, 0:P], axis=mybir.AxisListType.X)
                rr = smal.tile([P, 1], F32, tag="rr")
                nc.vector.reciprocal(rr, rcol)
                rr_cols.append(rr)

            if c + 1 < NCHUNK:
                Gp_next = emit_gp(TT_next)

            # ---- O = (Z^T.T @ Wv^T) * (1/r) + bv ----
            for qb in range(QC // P):
                ot = osb.tile([P, D], F32, tag="ot")
                for ec in range(2):
                    op_ps = mp.tile([P, 512], F32, tag="mp")
                    for db in range(DB):
                        nc.tensor.matmul(
                            op_ps,
                            Zs[:, db, qb * P:(qb + 1) * P],
                            WvT[:, db, ec * 512:(ec + 1) * 512],
                            start=(db == 0), stop=(db == DB - 1),
                        )
                    nc.vector.tensor_scalar_mul(
                        ot[:, ec * 512:(ec + 1) * 512], op_ps, rr_cols[qb])
                    nc.gpsimd.tensor_add(
                        ot[:, ec * 512:(ec + 1) * 512],
                        ot[:, ec * 512:(ec + 1) * 512],
                        bv_bc[:, ec * 512:(ec + 1) * 512])
                nc.scalar.dma_start(outp[q0 + qb * P:q0 + (qb + 1) * P, :], ot)
            TT, Gp = TT_next, Gp_next


def _get_nc():
    if "nc" not in _CACHE:
        _CACHE["nc"] = _build()
    return _CACHE["nc"]


def kernel(**inputs):
    inp = {k: np.ascontiguousarray(np.asarray(v, dtype=np.float32))
           for k, v in inputs.items()}
    nc = _get_nc()
    in_maps = [
        {
            "target": inp["target"][b],
            "non_target": inp["non_target"][b],
            "Wq": inp["Wq"], "Wk": inp["Wk"], "Wv": inp["Wv"],
            "bq": inp["bq"], "bv": inp["bv"],
        }
        for b in range(NCORES)
    ]
    res = run_bass_kernel_spmd(nc, in_maps, list(range(NCORES)),
                               trace=bool(int(os.environ.get("ATT_TRACE", "0"))))
    _CACHE["last_result"] = res
    out = np.stack([res.results[b]["out"] for b in range(NCORES)], axis=0)
    return out
